# revision 19
# baseline (speedup 1.0000x reference)
"""ConvTreeGRUCell on 8 Trainium2 NeuronCores.

Sharding: spatial over H. Each core owns 24 output rows (192/8) and
receives a 28-row input slab (2-row halo each side, zero-padded at the
image borders on the host). All three 3x3 convs, the per-child reset
gate, and the L-reductions are fully local per core - no collectives.

Per-core kernel (Tile framework), fp8 DoubleRow edition:
  - frames are 28 rows x 194 cols flattened to FREE=5434 (1 elem pad
    front/back). A conv tap (dy,dx) is the offset dy*194+dx.
  - conv = 5 accumulating fp8e4 DoubleRow matmuls per 512-px window:
    each DR matmul computes TWO taps (k-tiles) in one pass, the second
    tap addressed by a custom strided rhs AP (dim-1 stride = tap offset
    delta) over the same [child|x] fp8 cat tile. Tap 9 pairs with
    zeroed hi-weights. Weights are host-prescaled by 32 (fp8 range);
    the activation's scale=1/32 undoes it.
  - children processed in pairs: child 2p -> PSUM partitions 0..63,
    child 2p+1 -> 64..127 (tile_position col band). One joint
    128-partition sigmoid per window writes r2 (bf16).
  - vector path is bf16 on all 128 lanes: t2 = r2*c2 pair tiles,
    orh2 += t2; child_sum accumulated likewise. Partition folds
    (hi half + lo half) bounce through SBUF->SBUF DMA.
  - z/o convs run over fp8 [cs|x] and [rh|x] tiles, band 0 only;
    h = o + z*(cs - o) fused per window.
"""

import os
import sys

import numpy as np

for _p in ("/opt/trn_rl_repo",):
    if _p not in sys.path and os.path.isdir(_p):
        sys.path.insert(0, _p)

import ml_dtypes

import concourse.bass as bass
import bass_rust
import concourse.tile as tile
from concourse import bacc
from concourse import mybir
from concourse.bass_utils import run_bass_kernel_spmd

F32 = mybir.dt.float32
F8 = mybir.dt.float8e4
F16 = mybir.dt.float16
E4M3 = ml_dtypes.float8_e4m3

C = 64          # channels
L = 8           # children
HW = 192        # image H and W
NCORES = 8
OUT_ROWS = HW // NCORES          # 24 output rows per core
IN_ROWS = OUT_ROWS + 4           # 28-row slab (2-row halo each side)
WP = HW + 2                      # 194: padded row width
FRAME = IN_ROWS * WP             # 5432
FREE = FRAME + 2                 # 5434: +1 front pad, +1 tail pad

# flat index of (row r, col c) in the frame = 1 + r*WP + c
# stage 1 (r gate / reset_hidden): output rows 1..26
S1_LO = 1 + 1 * WP               # 195
S1_HI = 1 + 26 * WP + 194       # 5239 (exclusive)
S1N = S1_HI - S1_LO
# stage 2 (z / o / h): output rows 2..25
S2_LO = 1 + 2 * WP               # 389
S2_HI = 1 + 25 * WP + 194       # 5045 (exclusive)
S2N = S2_HI - S2_LO

NWIN = 512
SW = 32.0       # host-side weight prescale (undone by activation scale)

TAP_OFF = [dy * WP + dx for dy in (-1, 0, 1) for dx in (-1, 0, 1)]
# Dup-shift DoubleRow packing over [v | v<<1] tiles: each K=128 k-tile
# covers taps (o, o+1); each DR matmul covers 4 tap slots
# {o, o+1, o+d, o+d+1}. 3 matmuls cover all 9 taps (3 slots zeroed).
M_OFF = [-WP - 1, -WP + 1, 1]
M_DLT = [WP, 2 * WP - 2, WP]
NMM = 3


def _windows(lo, hi):
    out = []
    s = lo
    while s < hi:
        out.append((s, min(NWIN, hi - s)))
        s += NWIN
    return out


S1WIN = _windows(S1_LO, S1_HI)
S2WIN = _windows(S2_LO, S2_HI)

_BUILT = None


def _rhs(t, s, q, n):
    base = s + M_OFF[q]
    v = t[:, base:base + n].unsqueeze(1).broadcast_to([128, 2, n])
    pitch = v.ap[0][0]
    v.ap = bass_rust.VecI64Pair([[pitch, 128], [M_DLT[q], 2], [1, n]])
    return v


def _dr_conv(nc, ps_out, w, cat, s, n, stop=True):
    """One conv window: 3 DoubleRow matmuls accumulating into ps_out."""
    for q in range(NMM):
        nc.tensor.matmul(
            out=ps_out,
            lhsT=w[:, q, :, :],
            rhs=_rhs(cat, s, q, n),
            start=(q == 0),
            stop=(stop and q == NMM - 1),
            perf_mode=mybir.MatmulPerfMode.DoubleRow,
        )


def build_program():
    """Build the (SPMD, per-core) Bass program once."""
    nc = bacc.Bacc("TRN2")

    xdupd = nc.dram_tensor("xdupd", [2 * C, FREE], F8, kind="ExternalInput")
    x16d = nc.dram_tensor("x16d", [C, FREE], F16, kind="ExternalInput")
    c8d = nc.dram_tensor("c8d", [L, 2 * C, FREE], F8, kind="ExternalInput")
    cbd = nc.dram_tensor("cbd", [L, C, FREE], F16, kind="ExternalInput")
    wrcd = nc.dram_tensor("wrcd", [2 * C, NMM, 2, C], F8, kind="ExternalInput")
    ward = nc.dram_tensor("ward", [2 * C, NMM, 2, C], F8, kind="ExternalInput")
    identd = nc.dram_tensor("identd", [2 * C, C], F8, kind="ExternalInput")
    wzd = nc.dram_tensor("wzd", [2 * C, 9, C], F16, kind="ExternalInput")
    wod = nc.dram_tensor("wod", [2 * C, 9, C], F16, kind="ExternalInput")
    brd = nc.dram_tensor("brd", [2 * C, 1], F32, kind="ExternalInput")
    bzd = nc.dram_tensor("bzd", [C, 1], F32, kind="ExternalInput")
    bod = nc.dram_tensor("bod", [C, 1], F32, kind="ExternalInput")
    hout = nc.dram_tensor("hout", [C, OUT_ROWS, HW], F32, kind="ExternalOutput")

    sig = mybir.ActivationFunctionType.Sigmoid
    tanh = mybir.ActivationFunctionType.Tanh

    with tile.TileContext(nc) as tc:
        with (
            tc.tile_pool(name="singles", bufs=1) as singles,
            tc.tile_pool(name="c2p", bufs=4) as c2_pool,
            tc.tile_pool(name="r2p", bufs=2) as r2_pool,
            tc.tile_pool(name="t2p", bufs=2) as t2_pool,
            tc.tile_pool(name="hwin", bufs=3) as hwin_pool,
            tc.tile_pool(name="ps1", bufs=6, space="PSUM") as ps1_pool,
            tc.tile_pool(name="ps2", bufs=2, space="PSUM") as ps2_pool,
        ):
            # ---- persistent tiles ----
            cat = [singles.tile([2 * C, FREE], F8, tag=f"cat{i}", name=f"cat{i}")
                   for i in range(4)]
            wrc = singles.tile([2 * C, NMM, 2, C], F8, tag="wrc")
            war = singles.tile([2 * C, NMM, 2, C], F8, tag="war")
            ident = singles.tile([2 * C, C], F8, tag="ident")
            xdup = singles.tile([2 * C, FREE], F8, tag="xdup")
            a8 = singles.tile([2 * C, S1N], F8, tag="a8")
            wz = singles.tile([2 * C, 9, C], F16, tag="wz")
            wo = singles.tile([2 * C, 9, C], F16, tag="wo")
            br2 = singles.tile([2 * C, 1], F32, tag="br2")
            bz = singles.tile([C, 1], F32, tag="bz")
            bo = singles.tile([C, 1], F32, tag="bo")
            cs2 = singles.tile([2 * C, FREE], F16, tag="cs2")
            orh2 = singles.tile([2 * C, S1N], F16, tag="orh2")
            zs16 = singles.tile([2 * C, FREE], F16, tag="zs16")  # [cs | x]
            orh16 = singles.tile([2 * C, FREE], F16, tag="orh16")  # [rh | x]
            # csf and rhf share one buffer (disjoint lifetimes; the WAR
            # dep on the fold add serializes them)
            csf = singles.tile([C, FREE], F16, tag="csf")
            rhf = csf[:, 0:S1N]
            zb = singles.tile([C, S2N], F16, tag="zb")
            hb = singles.tile([C, S2N], F32, tag="hb")

            # ---- loads: strict priority order (DMA queues are in-order;
            # front-load is ~10MB vs ~29us of stage-1 headroom) ----
            nc.sync.dma_start(out=war, in_=ward[:])
            nc.sync.dma_start(out=xdup, in_=xdupd[:])
            nc.sync.dma_start(out=wrc, in_=wrcd[:])
            nc.sync.dma_start(out=ident, in_=identd[:])
            nc.sync.dma_start(out=br2, in_=brd[:])
            nc.sync.dma_start(out=cat[0], in_=c8d[0])
            nc.sync.dma_start(out=cat[1], in_=c8d[1])
            # pair-1 fp8 data before the fp16 bulk
            nc.sync.dma_start(out=cat[2], in_=c8d[2])
            nc.sync.dma_start(out=cat[3], in_=c8d[3])
            # fp16 children (child_sum chain + r*c muls)
            c2t = []
            for p in range(4):
                c2 = c2_pool.tile([2 * C, FREE], F16, tag="c2")
                c2t.append(c2)
                nc.sync.dma_start(out=c2[0:C, :], in_=cbd[2 * p])
                nc.sync.dma_start(out=c2[C:2 * C, :], in_=cbd[2 * p + 1])
            # stage-2 weights/biases/x copies (needed last)
            nc.sync.dma_start(out=wz, in_=wzd[:])
            nc.sync.dma_start(out=wo, in_=wod[:])
            nc.sync.dma_start(out=bz, in_=bzd[:])
            nc.sync.dma_start(out=bo, in_=bod[:])
            nc.sync.dma_start(out=zs16[C:2 * C, :], in_=x16d[:])
            nc.sync.dma_start(out=orh16[C:2 * C, :], in_=x16d[:])
            # rh rows 0/27 and flat pads are read by the o-conv but never
            # written by the fold below; zero them once.
            nc.gpsimd.memset(a8[C:2 * C, :], 0.0)
            nc.gpsimd.memset(orh16[0:C, 0:S1_LO], 0.0)
            nc.gpsimd.memset(orh16[0:C, S1_HI:FREE], 0.0)

            # ---- stage 1: per-child-pair reset gate + reductions ----
            # vector issue order matters (in-order queue): all child_sum
            # accumulation goes ahead of the r2-gated muls so the cs fold
            # (and the z-conv behind it) never waits for stage-1 sigmoids.
            # mul0 is interleaved before cs+=c2[3] because c2[3] reuses
            # c2[0]'s pool buffer (bufs=3) - anything later would deadlock
            # the in-order queue on the WAR dependency.
            # shared x-part of the reset gate: a = conv(x, Wr_x), once,
            # prescaled by SW, stored fp8 for the per-child identity-add
            for s, n in S1WIN:
                psx = ps1_pool.tile([C, NWIN], F32, tag="ps")
                _dr_conv(nc, psx[:, :n], war, xdup, s, n)
                nc.scalar.activation(
                    out=a8[0:C, s - S1_LO:s - S1_LO + n], in_=psx[:, :n],
                    func=mybir.ActivationFunctionType.Copy,
                )

            r2t = []
            for p in range(4):
                a, b = 2 * p, 2 * p + 1
                cat_a, cat_b = cat[a % 4], cat[b % 4]
                c2 = c2t[p]

                r2 = r2_pool.tile([2 * C, S1N], F16, tag="r2")
                r2t.append(r2)
                for s, n in S1WIN:
                    j = s - S1_LO
                    psa = ps1_pool.tile([C, NWIN], F32, tag="ps")
                    psb = ps1_pool.tile([C, NWIN], F32, tag="ps")
                    for ps, ct in ((psa, cat_a), (psb, cat_b)):
                        _dr_conv(nc, ps[:, :n], wrc, ct, s, n, stop=False)
                        nc.tensor.matmul(
                            out=ps[:, :n],
                            lhsT=ident,
                            rhs=a8[:, j:j + n],
                            start=False,
                            stop=True,
                        )
                    nc.scalar.activation(
                        out=r2[0:C, j:j + n], in_=psa[:, :n], func=sig,
                        bias=br2[0:C, 0:1], scale=1.0 / SW,
                    )
                    nc.scalar.activation(
                        out=r2[C:2 * C, j:j + n], in_=psb[:, :n], func=sig,
                        bias=br2[0:C, 0:1], scale=1.0 / SW,
                    )
                # refill this pair's fp8 cat tiles for pair p+2
                if p < 2:
                    nc.sync.dma_start(out=cat_a, in_=c8d[a + 4])
                    nc.sync.dma_start(out=cat_b, in_=c8d[b + 4])

                # child_sum accumulation (fp16, 128 lanes), early
                if p == 0:
                    nc.vector.tensor_copy(out=cs2, in_=c2)
                elif p < 3:
                    nc.vector.tensor_add(out=cs2, in0=cs2, in1=c2)
                if p == 2:
                    # mul0 here: c2[3] reuses c2[0]'s buffer (WAR) and
                    # r2[2] reuses r2[0]'s
                    nc.vector.tensor_mul(
                        out=orh2, in0=r2t[0], in1=c2t[0][:, S1_LO:S1_HI]
                    )
                if p == 3:
                    nc.vector.tensor_add(out=cs2, in0=cs2, in1=c2)
                    # fold child_sum into zs16 lo now so the z-conv can
                    # start the moment stage-1 matmuls drain
                    nc.sync.dma_start(out=csf, in_=cs2[C:2 * C, :])
                    nc.vector.tensor_add(
                        out=zs16[0:C, :], in0=cs2[0:C, :], in1=csf
                    )
                    # mul1 before pair-3 sigmoids overwrite r2[1]'s buffer
                    t2 = t2_pool.tile([2 * C, S1N], F16, tag="t2")
                    nc.vector.tensor_mul(
                        out=t2, in0=r2t[1], in1=c2t[1][:, S1_LO:S1_HI]
                    )
                    nc.vector.tensor_add(out=orh2, in0=orh2, in1=t2)
            # remaining reset_hidden accumulation (fp16, 128 lanes)
            for p in (2, 3):
                t2 = t2_pool.tile([2 * C, S1N], F16, tag="t2")
                nc.vector.tensor_mul(
                    out=t2, in0=r2t[p], in1=c2t[p][:, S1_LO:S1_HI]
                )
                nc.vector.tensor_add(out=orh2, in0=orh2, in1=t2)

            # ---- reset_hidden partition fold (hi half + lo half) ----
            nc.sync.dma_start(out=rhf, in_=orh2[C:2 * C, :])
            nc.vector.tensor_add(
                out=orh16[0:C, S1_LO:S1_HI], in0=orh2[0:C, :], in1=rhf
            )

            # ---- stage 2: z-conv windows, then o-conv + fused h ----
            for s, n in S2WIN:
                psz = ps2_pool.tile([C, NWIN], F32, tag="ps2")
                for t in range(9):
                    o = TAP_OFF[t]
                    nc.tensor.matmul(
                        out=psz[:, :n],
                        lhsT=wz[:, t, :],
                        rhs=zs16[:, s + o:s + o + n],
                        start=(t == 0),
                        stop=(t == 8),
                    )
                nc.scalar.activation(
                    out=zb[:, s - S2_LO:s - S2_LO + n],
                    in_=psz[:, :n],
                    func=sig,
                    bias=bz[:, 0:1],
                )

            for s, n in S2WIN:
                j = s - S2_LO
                pso = ps2_pool.tile([C, NWIN], F32, tag="ps2")
                for t in range(9):
                    o = TAP_OFF[t]
                    nc.tensor.matmul(
                        out=pso[:, :n],
                        lhsT=wo[:, t, :],
                        rhs=orh16[:, s + o:s + o + n],
                        start=(t == 0),
                        stop=(t == 8),
                    )
                ow = hwin_pool.tile([C, NWIN], F16, tag="ow")
                t1w = hwin_pool.tile([C, NWIN], F16, tag="t1w")
                nc.scalar.activation(
                    out=ow[:, :n], in_=pso[:, :n], func=tanh,
                    bias=bo[:, 0:1],
                )
                # h_w = o_w + z_w * (cs_w - o_w)
                nc.vector.scalar_tensor_tensor(
                    out=t1w[:, :n],
                    in0=ow[:, :n],
                    scalar=-1.0,
                    in1=zs16[0:C, s:s + n],
                    op0=mybir.AluOpType.mult,
                    op1=mybir.AluOpType.add,
                )
                nc.vector.tensor_mul(
                    out=t1w[:, :n], in0=zb[:, j:j + n], in1=t1w[:, :n]
                )
                nc.vector.tensor_add(
                    out=hb[:, j:j + n], in0=ow[:, :n], in1=t1w[:, :n]
                )

            # ---- store: drop the pad column of each row; 3 chunks so the
            # store overlaps the last blend windows ----
            htr = hb.rearrange("p (r w) -> p r w", w=WP)
            for r0 in range(0, OUT_ROWS, 8):
                nc.sync.dma_start(
                    out=hout[:, r0:r0 + 8, :],
                    in_=htr[:, r0:r0 + 8, 1:HW + 1],
                )

    nc.finalize()
    return nc


def _get_program():
    global _BUILT
    if _BUILT is None:
        _BUILT = build_program()
    return _BUILT


def make_in_maps(x, child_h, Wr, br, Wz, bz, Wo, bo):
    """Host-side sharding: pad borders/columns, slice 28-row slabs,
    convert to fp8/bf16, and prepack DoubleRow weights."""
    x = np.asarray(x, dtype=np.float32)
    child_h = np.asarray(child_h, dtype=np.float32)

    # zero-pad H by 2 (halo at image border) and W by 1 (conv column pad)
    xp = np.zeros((C, HW + 4, WP), dtype=np.float32)
    xp[:, 2:2 + HW, 1:1 + HW] = x[0]
    cp = np.zeros((L, C, HW + 4, WP), dtype=np.float32)
    cp[:, :, 2:2 + HW, 1:1 + HW] = child_h[:, 0]

    def frame(a):  # [..., IN_ROWS, WP] -> [..., FREE] with front/tail pad
        flat = a.reshape(a.shape[:-2] + (FRAME,))
        out = np.zeros(a.shape[:-2] + (FREE,), dtype=np.float32)
        out[..., 1:1 + FRAME] = flat
        return out

    def wprep(w, part):
        # [C, 2C, 3, 3] -> [2C, NMM, 2, C] dup-shift DoubleRow slots,
        # prescaled. part selects the x half (0:C) or child half (C:2C)
        # of the reference input channels.
        wt = np.transpose(np.asarray(w, np.float32), (1, 2, 3, 0)).reshape(
            2 * C, 9, C)[part] * SW  # [C, 9, C]
        off2tap = {TAP_OFF[t]: t for t in range(9)}
        wp_ = np.zeros((2 * C, NMM, 2, C), dtype=np.float32)
        for m in range(NMM):
            for i in range(2):
                o = M_OFF[m] + i * M_DLT[m]
                if o in off2tap:
                    wp_[0:C, m, i, :] = wt[:, off2tap[o], :]
                if o + 1 in off2tap:
                    wp_[C:2 * C, m, i, :] = wt[:, off2tap[o + 1], :]
        return np.ascontiguousarray(wp_).astype(E4M3)

    def dup(a):  # [..., ch, FREE] -> [..., 2ch, FREE] with [v | v<<1]
        sh = np.zeros_like(a)
        sh[..., :, 0:FREE - 1] = a[..., :, 1:FREE]
        return np.concatenate([a, sh], axis=-2)

    def wprep16(w):
        wt = np.transpose(np.asarray(w, np.float32), (1, 2, 3, 0)).reshape(
            2 * C, 9, C)
        wt = np.concatenate([wt[C:], wt[:C]], axis=0)
        return np.ascontiguousarray(wt).astype(np.float16)

    wrct = wprep(Wr, slice(C, 2 * C))
    wart = wprep(Wr, slice(0, C))
    wzt, wot = wprep16(Wz), wprep16(Wo)
    ident = np.concatenate(
        [np.eye(C, dtype=np.float32), np.zeros((C, C), np.float32)]
    ).astype(E4M3)
    br2 = np.concatenate(
        [np.asarray(br, np.float32)] * 2).reshape(2 * C, 1)
    bzt = np.asarray(bz, np.float32).reshape(C, 1)
    bot = np.asarray(bo, np.float32).reshape(C, 1)

    in_maps = []
    for k in range(NCORES):
        r0 = k * OUT_ROWS  # global output row start; slab = rows r0-2 .. r0+26
        xf = frame(xp[:, r0:r0 + IN_ROWS, :])
        cf = frame(cp[:, :, r0:r0 + IN_ROWS, :])
        in_maps.append({
            "xdupd": dup(xf.astype(E4M3)),
            "x16d": xf.astype(np.float16),
            "c8d": dup(cf.astype(E4M3)),
            "cbd": cf.astype(np.float16),
            "wrcd": wrct, "ward": wart, "identd": ident,
            "wzd": wzt, "wod": wot,
            "brd": br2, "bzd": bzt, "bod": bot,
        })
    return in_maps


def run(in_maps, trace=False):
    nc = _get_program()
    return run_bass_kernel_spmd(nc, in_maps, list(range(NCORES)), trace=trace)


def kernel(x, child_h, Wr, br, Wz, bz, Wo, bo):
    in_maps = make_in_maps(x, child_h, Wr, br, Wz, bz, Wo, bo)
    res = run(in_maps).results
    out = np.empty((1, C, HW, HW), dtype=np.float32)
    for k in range(NCORES):
        out[0, :, k * OUT_ROWS:(k + 1) * OUT_ROWS, :] = res[k]["hout"]
    return out


# revision 20
# speedup vs baseline: 1.0974x; 1.0974x over previous
"""ConvTreeGRUCell on 8 Trainium2 NeuronCores.

Sharding: spatial over H. Each core owns 24 output rows (192/8) and
receives a 28-row input slab (2-row halo each side, zero-padded at the
image borders on the host). All three 3x3 convs, the per-child reset
gate, and the L-reductions are fully local per core - no collectives.

Per-core kernel (Tile framework), fp8 DoubleRow edition:
  - frames are 28 rows x 194 cols flattened to FREE=5434 (1 elem pad
    front/back). A conv tap (dy,dx) is the offset dy*194+dx.
  - conv = 5 accumulating fp8e4 DoubleRow matmuls per 512-px window:
    each DR matmul computes TWO taps (k-tiles) in one pass, the second
    tap addressed by a custom strided rhs AP (dim-1 stride = tap offset
    delta) over the same [child|x] fp8 cat tile. Tap 9 pairs with
    zeroed hi-weights. Weights are host-prescaled by 32 (fp8 range);
    the activation's scale=1/32 undoes it.
  - children processed in pairs: child 2p -> PSUM partitions 0..63,
    child 2p+1 -> 64..127 (tile_position col band). One joint
    128-partition sigmoid per window writes r2 (bf16).
  - vector path is bf16 on all 128 lanes: t2 = r2*c2 pair tiles,
    orh2 += t2; child_sum accumulated likewise. Partition folds
    (hi half + lo half) bounce through SBUF->SBUF DMA.
  - z/o convs run over fp8 [cs|x] and [rh|x] tiles, band 0 only;
    h = o + z*(cs - o) fused per window.
"""

import os
import sys

import numpy as np

for _p in ("/opt/trn_rl_repo",):
    if _p not in sys.path and os.path.isdir(_p):
        sys.path.insert(0, _p)

import ml_dtypes

import concourse.bass as bass
import bass_rust
import concourse.tile as tile
from concourse import bacc
from concourse import mybir
from concourse.bass_utils import run_bass_kernel_spmd

F32 = mybir.dt.float32
F8 = mybir.dt.float8e4
F16 = mybir.dt.float16
E4M3 = ml_dtypes.float8_e4m3

C = 64          # channels
L = 8           # children
HW = 192        # image H and W
NCORES = 8
OUT_ROWS = HW // NCORES          # 24 output rows per core
IN_ROWS = OUT_ROWS + 4           # 28-row slab (2-row halo each side)
WP = HW + 2                      # 194: padded row width
FRAME = IN_ROWS * WP             # 5432
FREE = FRAME + 2                 # 5434: +1 front pad, +1 tail pad

# flat index of (row r, col c) in the frame = 1 + r*WP + c
# stage 1 (r gate / reset_hidden): output rows 1..26
S1_LO = 1 + 1 * WP               # 195
S1_HI = 1 + 26 * WP + 194       # 5239 (exclusive)
S1N = S1_HI - S1_LO
# stage 2 (z / o / h): output rows 2..25
S2_LO = 1 + 2 * WP               # 389
S2_HI = 1 + 25 * WP + 194       # 5045 (exclusive)
S2N = S2_HI - S2_LO

NWIN = 512
SW = 32.0       # host-side weight prescale (undone by activation scale)

TAP_OFF = [dy * WP + dx for dy in (-1, 0, 1) for dx in (-1, 0, 1)]
# Dup-shift DoubleRow packing over [v | v<<1] tiles: each K=128 k-tile
# covers taps (o, o+1); each DR matmul covers 4 tap slots
# {o, o+1, o+d, o+d+1}. 3 matmuls cover all 9 taps (3 slots zeroed).
M_OFF = [-WP - 1, -WP + 1, 1]
M_DLT = [WP, 2 * WP - 2, WP]
NMM = 3


def _windows(lo, hi):
    out = []
    s = lo
    while s < hi:
        out.append((s, min(NWIN, hi - s)))
        s += NWIN
    return out


S1WIN = _windows(S1_LO, S1_HI)
S2WIN = _windows(S2_LO, S2_HI)

_BUILT = None


def _rhs(t, s, q, n):
    base = s + M_OFF[q]
    v = t[:, base:base + n].unsqueeze(1).broadcast_to([128, 2, n])
    pitch = v.ap[0][0]
    v.ap = bass_rust.VecI64Pair([[pitch, 128], [M_DLT[q], 2], [1, n]])
    return v


def _dr_conv(nc, ps_out, w, cat, s, n, stop=True):
    """One conv window: 3 DoubleRow matmuls accumulating into ps_out."""
    for q in range(NMM):
        nc.tensor.matmul(
            out=ps_out,
            lhsT=w[:, q, :, :],
            rhs=_rhs(cat, s, q, n),
            start=(q == 0),
            stop=(stop and q == NMM - 1),
            perf_mode=mybir.MatmulPerfMode.DoubleRow,
        )


def build_program():
    """Build the (SPMD, per-core) Bass program once."""
    nc = bacc.Bacc("TRN2")

    xdupd = nc.dram_tensor("xdupd", [2 * C, FREE], F8, kind="ExternalInput")
    x16d = nc.dram_tensor("x16d", [C, FREE], F16, kind="ExternalInput")
    c8d = nc.dram_tensor("c8d", [L, 2 * C, FREE], F8, kind="ExternalInput")
    cbd = nc.dram_tensor("cbd", [L, C, FREE], F16, kind="ExternalInput")
    wrcd = nc.dram_tensor("wrcd", [2 * C, NMM, 2, C], F8, kind="ExternalInput")
    ward = nc.dram_tensor("ward", [2 * C, NMM, 2, C], F8, kind="ExternalInput")
    identd = nc.dram_tensor("identd", [2 * C, 2, C], F8, kind="ExternalInput")
    wzd = nc.dram_tensor("wzd", [2 * C, 9, C], F16, kind="ExternalInput")
    wod = nc.dram_tensor("wod", [2 * C, 9, C], F16, kind="ExternalInput")
    brd = nc.dram_tensor("brd", [2 * C, 1], F32, kind="ExternalInput")
    bzd = nc.dram_tensor("bzd", [C, 1], F32, kind="ExternalInput")
    bod = nc.dram_tensor("bod", [C, 1], F32, kind="ExternalInput")
    hout = nc.dram_tensor("hout", [C, OUT_ROWS, HW], F32, kind="ExternalOutput")

    sig = mybir.ActivationFunctionType.Sigmoid
    tanh = mybir.ActivationFunctionType.Tanh

    with tile.TileContext(nc) as tc:
        with (
            tc.tile_pool(name="singles", bufs=1) as singles,
            tc.tile_pool(name="c2p", bufs=4) as c2_pool,
            tc.tile_pool(name="r2p", bufs=2) as r2_pool,
            tc.tile_pool(name="t2p", bufs=2) as t2_pool,
            tc.tile_pool(name="hwin", bufs=3) as hwin_pool,
            tc.tile_pool(name="ps1", bufs=6, space="PSUM") as ps1_pool,
            tc.tile_pool(name="ps2", bufs=2, space="PSUM") as ps2_pool,
        ):
            # ---- persistent tiles ----
            cat = [singles.tile([2 * C, FREE], F8, tag=f"cat{i}", name=f"cat{i}")
                   for i in range(4)]
            wrc = singles.tile([2 * C, NMM, 2, C], F8, tag="wrc")
            war = singles.tile([2 * C, NMM, 2, C], F8, tag="war")
            ident = singles.tile([2 * C, 2, C], F8, tag="ident")
            xdup = singles.tile([2 * C, FREE], F8, tag="xdup")
            a8 = singles.tile([2 * C, S1N], F8, tag="a8")
            wz = singles.tile([2 * C, 9, C], F16, tag="wz")
            wo = singles.tile([2 * C, 9, C], F16, tag="wo")
            br2 = singles.tile([2 * C, 1], F32, tag="br2")
            bz = singles.tile([C, 1], F32, tag="bz")
            bo = singles.tile([C, 1], F32, tag="bo")
            cs2 = singles.tile([2 * C, FREE], F16, tag="cs2")
            orh2 = singles.tile([2 * C, S1N], F16, tag="orh2")
            zs16 = singles.tile([2 * C, FREE], F16, tag="zs16")  # [cs | x]
            orh16 = singles.tile([2 * C, FREE], F16, tag="orh16")  # [rh | x]
            # csf and rhf share one buffer (disjoint lifetimes; the WAR
            # dep on the fold add serializes them)
            csf = singles.tile([C, FREE], F16, tag="csf")
            rhf = csf[:, 0:S1N]
            zb = singles.tile([C, S2N], F16, tag="zb")
            hb = singles.tile([C, S2N], F32, tag="hb")

            # ---- loads: strict priority order (DMA queues are in-order;
            # front-load is ~10MB vs ~29us of stage-1 headroom) ----
            nc.sync.dma_start(out=war, in_=ward[:])
            nc.sync.dma_start(out=xdup, in_=xdupd[:])
            nc.sync.dma_start(out=wrc, in_=wrcd[:])
            nc.sync.dma_start(out=ident, in_=identd[:])
            nc.sync.dma_start(out=br2, in_=brd[:])
            nc.sync.dma_start(out=cat[0], in_=c8d[0])
            nc.sync.dma_start(out=cat[1], in_=c8d[1])
            # pair-1 fp8 data before the fp16 bulk
            nc.sync.dma_start(out=cat[2], in_=c8d[2])
            nc.sync.dma_start(out=cat[3], in_=c8d[3])
            # fp16 children (child_sum chain + r*c muls)
            c2t = []
            for p in range(4):
                c2 = c2_pool.tile([2 * C, FREE], F16, tag="c2")
                c2t.append(c2)
                nc.sync.dma_start(out=c2[0:C, :], in_=cbd[2 * p])
                nc.sync.dma_start(out=c2[C:2 * C, :], in_=cbd[2 * p + 1])
            # stage-2 weights/biases/x copies (needed last)
            nc.sync.dma_start(out=wz, in_=wzd[:])
            nc.sync.dma_start(out=wo, in_=wod[:])
            nc.sync.dma_start(out=bz, in_=bzd[:])
            nc.sync.dma_start(out=bo, in_=bod[:])
            nc.sync.dma_start(out=zs16[C:2 * C, :], in_=x16d[:])
            nc.sync.dma_start(out=orh16[C:2 * C, :], in_=x16d[:])
            # rh rows 0/27 and flat pads are read by the o-conv but never
            # written by the fold below; zero them once.
            nc.gpsimd.memset(a8[C:2 * C, :], 0.0)
            nc.gpsimd.memset(orh16[0:C, 0:S1_LO], 0.0)
            nc.gpsimd.memset(orh16[0:C, S1_HI:FREE], 0.0)

            # ---- stage 1: per-child-pair reset gate + reductions ----
            # vector issue order matters (in-order queue): all child_sum
            # accumulation goes ahead of the r2-gated muls so the cs fold
            # (and the z-conv behind it) never waits for stage-1 sigmoids.
            # mul0 is interleaved before cs+=c2[3] because c2[3] reuses
            # c2[0]'s pool buffer (bufs=3) - anything later would deadlock
            # the in-order queue on the WAR dependency.
            # shared x-part of the reset gate: a = conv(x, Wr_x), once,
            # prescaled by SW, stored fp8 for the per-child identity-add
            for s, n in S1WIN:
                psx = ps1_pool.tile([C, NWIN], F32, tag="ps")
                _dr_conv(nc, psx[:, :n], war, xdup, s, n)
                nc.scalar.activation(
                    out=a8[0:C, s - S1_LO:s - S1_LO + n], in_=psx[:, :n],
                    func=mybir.ActivationFunctionType.Copy,
                )

            r2t = []
            for p in range(4):
                a, b = 2 * p, 2 * p + 1
                cat_a, cat_b = cat[a % 4], cat[b % 4]
                c2 = c2t[p]

                r2 = r2_pool.tile([2 * C, S1N], F16, tag="r2")
                r2t.append(r2)
                for s, n in S1WIN:
                    j = s - S1_LO
                    psa = ps1_pool.tile([C, NWIN], F32, tag="ps")
                    psb = ps1_pool.tile([C, NWIN], F32, tag="ps")
                    for ps, ct in ((psa, cat_a), (psb, cat_b)):
                        _dr_conv(nc, ps[:, :n], wrc, ct, s, n, stop=False)
                        va = a8[:, j:j + n].unsqueeze(1).broadcast_to(
                            [128, 2, n])
                        va.ap = bass_rust.VecI64Pair(
                            [[va.ap[0][0], 128], [0, 2], [1, n]])
                        nc.tensor.matmul(
                            out=ps[:, :n],
                            lhsT=ident,
                            rhs=va,
                            start=False,
                            stop=True,
                            perf_mode=mybir.MatmulPerfMode.DoubleRow,
                        )
                    nc.scalar.activation(
                        out=r2[0:C, j:j + n], in_=psa[:, :n], func=sig,
                        bias=br2[0:C, 0:1], scale=1.0 / SW,
                    )
                    nc.scalar.activation(
                        out=r2[C:2 * C, j:j + n], in_=psb[:, :n], func=sig,
                        bias=br2[0:C, 0:1], scale=1.0 / SW,
                    )
                # refill this pair's fp8 cat tiles for pair p+2
                if p < 2:
                    nc.sync.dma_start(out=cat_a, in_=c8d[a + 4])
                    nc.sync.dma_start(out=cat_b, in_=c8d[b + 4])

                # child_sum accumulation (fp16, 128 lanes), early
                if p == 0:
                    nc.vector.tensor_copy(out=cs2, in_=c2)
                elif p < 3:
                    nc.vector.tensor_add(out=cs2, in0=cs2, in1=c2)
                if p == 2:
                    # mul0 here: c2[3] reuses c2[0]'s buffer (WAR) and
                    # r2[2] reuses r2[0]'s
                    nc.vector.tensor_mul(
                        out=orh2, in0=r2t[0], in1=c2t[0][:, S1_LO:S1_HI]
                    )
                if p == 3:
                    nc.vector.tensor_add(out=cs2, in0=cs2, in1=c2)
                    # fold child_sum into zs16 lo now so the z-conv can
                    # start the moment stage-1 matmuls drain
                    nc.sync.dma_start(out=csf, in_=cs2[C:2 * C, :])
                    nc.vector.tensor_add(
                        out=zs16[0:C, :], in0=cs2[0:C, :], in1=csf
                    )
                    # mul1 before pair-3 sigmoids overwrite r2[1]'s buffer
                    t2 = t2_pool.tile([2 * C, S1N], F16, tag="t2")
                    nc.vector.tensor_mul(
                        out=t2, in0=r2t[1], in1=c2t[1][:, S1_LO:S1_HI]
                    )
                    nc.vector.tensor_add(out=orh2, in0=orh2, in1=t2)
            # remaining reset_hidden accumulation (fp16, 128 lanes)
            for p in (2, 3):
                t2 = t2_pool.tile([2 * C, S1N], F16, tag="t2")
                nc.vector.tensor_mul(
                    out=t2, in0=r2t[p], in1=c2t[p][:, S1_LO:S1_HI]
                )
                nc.vector.tensor_add(out=orh2, in0=orh2, in1=t2)

            # ---- reset_hidden partition fold (hi half + lo half) ----
            nc.sync.dma_start(out=rhf, in_=orh2[C:2 * C, :])
            nc.vector.tensor_add(
                out=orh16[0:C, S1_LO:S1_HI], in0=orh2[0:C, :], in1=rhf
            )

            # ---- stage 2: z-conv windows, then o-conv + fused h ----
            for s, n in S2WIN:
                psz = ps2_pool.tile([C, NWIN], F32, tag="ps2")
                for t in range(9):
                    o = TAP_OFF[t]
                    nc.tensor.matmul(
                        out=psz[:, :n],
                        lhsT=wz[:, t, :],
                        rhs=zs16[:, s + o:s + o + n],
                        start=(t == 0),
                        stop=(t == 8),
                    )
                nc.scalar.activation(
                    out=zb[:, s - S2_LO:s - S2_LO + n],
                    in_=psz[:, :n],
                    func=sig,
                    bias=bz[:, 0:1],
                )

            for s, n in S2WIN:
                j = s - S2_LO
                pso = ps2_pool.tile([C, NWIN], F32, tag="ps2")
                for t in range(9):
                    o = TAP_OFF[t]
                    nc.tensor.matmul(
                        out=pso[:, :n],
                        lhsT=wo[:, t, :],
                        rhs=orh16[:, s + o:s + o + n],
                        start=(t == 0),
                        stop=(t == 8),
                    )
                ow = hwin_pool.tile([C, NWIN], F16, tag="ow")
                t1w = hwin_pool.tile([C, NWIN], F16, tag="t1w")
                nc.scalar.activation(
                    out=ow[:, :n], in_=pso[:, :n], func=tanh,
                    bias=bo[:, 0:1],
                )
                # h_w = o_w + z_w * (cs_w - o_w)
                nc.vector.scalar_tensor_tensor(
                    out=t1w[:, :n],
                    in0=ow[:, :n],
                    scalar=-1.0,
                    in1=zs16[0:C, s:s + n],
                    op0=mybir.AluOpType.mult,
                    op1=mybir.AluOpType.add,
                )
                nc.vector.tensor_mul(
                    out=t1w[:, :n], in0=zb[:, j:j + n], in1=t1w[:, :n]
                )
                nc.vector.tensor_add(
                    out=hb[:, j:j + n], in0=ow[:, :n], in1=t1w[:, :n]
                )

            # ---- store: drop the pad column of each row; 3 chunks so the
            # store overlaps the last blend windows ----
            htr = hb.rearrange("p (r w) -> p r w", w=WP)
            for r0 in range(0, OUT_ROWS, 8):
                nc.sync.dma_start(
                    out=hout[:, r0:r0 + 8, :],
                    in_=htr[:, r0:r0 + 8, 1:HW + 1],
                )

    nc.finalize()
    return nc


def _get_program():
    global _BUILT
    if _BUILT is None:
        _BUILT = build_program()
    return _BUILT


def make_in_maps(x, child_h, Wr, br, Wz, bz, Wo, bo):
    """Host-side sharding: pad borders/columns, slice 28-row slabs,
    convert to fp8/bf16, and prepack DoubleRow weights."""
    x = np.asarray(x, dtype=np.float32)
    child_h = np.asarray(child_h, dtype=np.float32)

    # zero-pad H by 2 (halo at image border) and W by 1 (conv column pad)
    xp = np.zeros((C, HW + 4, WP), dtype=np.float32)
    xp[:, 2:2 + HW, 1:1 + HW] = x[0]
    cp = np.zeros((L, C, HW + 4, WP), dtype=np.float32)
    cp[:, :, 2:2 + HW, 1:1 + HW] = child_h[:, 0]

    def frame(a):  # [..., IN_ROWS, WP] -> [..., FREE] with front/tail pad
        flat = a.reshape(a.shape[:-2] + (FRAME,))
        out = np.zeros(a.shape[:-2] + (FREE,), dtype=np.float32)
        out[..., 1:1 + FRAME] = flat
        return out

    def wprep(w, part):
        # [C, 2C, 3, 3] -> [2C, NMM, 2, C] dup-shift DoubleRow slots,
        # prescaled. part selects the x half (0:C) or child half (C:2C)
        # of the reference input channels.
        wt = np.transpose(np.asarray(w, np.float32), (1, 2, 3, 0)).reshape(
            2 * C, 9, C)[part] * SW  # [C, 9, C]
        off2tap = {TAP_OFF[t]: t for t in range(9)}
        wp_ = np.zeros((2 * C, NMM, 2, C), dtype=np.float32)
        for m in range(NMM):
            for i in range(2):
                o = M_OFF[m] + i * M_DLT[m]
                if o in off2tap:
                    wp_[0:C, m, i, :] = wt[:, off2tap[o], :]
                if o + 1 in off2tap:
                    wp_[C:2 * C, m, i, :] = wt[:, off2tap[o + 1], :]
        return np.ascontiguousarray(wp_).astype(E4M3)

    def dup(a):  # [..., ch, FREE] -> [..., 2ch, FREE] with [v | v<<1]
        sh = np.zeros_like(a)
        sh[..., :, 0:FREE - 1] = a[..., :, 1:FREE]
        return np.concatenate([a, sh], axis=-2)

    def wprep16(w):
        wt = np.transpose(np.asarray(w, np.float32), (1, 2, 3, 0)).reshape(
            2 * C, 9, C)
        wt = np.concatenate([wt[C:], wt[:C]], axis=0)
        return np.ascontiguousarray(wt).astype(np.float16)

    wrct = wprep(Wr, slice(C, 2 * C))
    wart = wprep(Wr, slice(0, C))
    wzt, wot = wprep16(Wz), wprep16(Wo)
    ident = np.zeros((2 * C, 2, C), np.float32)
    ident[0:C, 0, :] = np.eye(C, dtype=np.float32)
    ident = ident.astype(E4M3)
    br2 = np.concatenate(
        [np.asarray(br, np.float32)] * 2).reshape(2 * C, 1)
    bzt = np.asarray(bz, np.float32).reshape(C, 1)
    bot = np.asarray(bo, np.float32).reshape(C, 1)

    in_maps = []
    for k in range(NCORES):
        r0 = k * OUT_ROWS  # global output row start; slab = rows r0-2 .. r0+26
        xf = frame(xp[:, r0:r0 + IN_ROWS, :])
        cf = frame(cp[:, :, r0:r0 + IN_ROWS, :])
        in_maps.append({
            "xdupd": dup(xf.astype(E4M3)),
            "x16d": xf.astype(np.float16),
            "c8d": dup(cf.astype(E4M3)),
            "cbd": cf.astype(np.float16),
            "wrcd": wrct, "ward": wart, "identd": ident,
            "wzd": wzt, "wod": wot,
            "brd": br2, "bzd": bzt, "bod": bot,
        })
    return in_maps


def run(in_maps, trace=False):
    nc = _get_program()
    return run_bass_kernel_spmd(nc, in_maps, list(range(NCORES)), trace=trace)


def kernel(x, child_h, Wr, br, Wz, bz, Wo, bo):
    in_maps = make_in_maps(x, child_h, Wr, br, Wz, bz, Wo, bo)
    res = run(in_maps).results
    out = np.empty((1, C, HW, HW), dtype=np.float32)
    for k in range(NCORES):
        out[0, :, k * OUT_ROWS:(k + 1) * OUT_ROWS, :] = res[k]["hout"]
    return out
